# revision 14
# baseline (speedup 1.0000x reference)
"""Causal GQA self-attention (B=2, S=2048, D=2048, 16 heads / 4 KV heads) on 8
Trainium2 NeuronCores.

Sharding: (kv-head x batch). Core c owns kv head c//2 and batch c%2: it
projects the 4 GQA query heads of that kv head plus k,v for its batch's 2048
tokens (no duplicated k/v work), runs causal attention for those 4 heads, and
computes the partial output projection for its batch's tokens. The host sums
the 4 kv-group partials per batch.

Device-side design (carried over from the head-TP version):
  - All matmuls bf16 (PSUM f32); x/weights/outputs cross DMA as bf16.
  - Scores are computed transposed: sc[key, query] = kT_tile^T @ qT_group,
    feeding AV directly with no PE transposes. The softmax denominator comes
    from a ones[128,128]-stationary matmul -> lb[128,512] (128 replicated
    rows) -> single full-width reciprocal_approx_fast.
  - QKV projection runs in two passes per token tile (q: 512 wide, kv: 256
    wide) to respect PSUM bank limits; RMS+RoPE is software-pipelined so the
    PE queue never waits on the DVE/Pool rope chain.
"""

import math

import numpy as np

B = 2
S = 2048
D = 2048
T = B * S
NH = 16
NKV = 4
HD = 128
P = 128
ROPE_BASE = 10000.0
EPS = float(np.finfo(np.float32).eps)
NEG = -1.0e30

N_CORES = 8
TT_B = S // P          # 16 token tiles per batch
GROUPS = 4             # groups of 4 q-tiles (512 queries)
QW = 512               # q projection width (4 heads)
KVW = 256              # k+v projection width
NSEG = 5               # rms/rope segments: q0..q3, k
HB = 8                 # token tiles per rsqrt batch

_PROG = {}

# tuning flags (A/B'd via TimelineSim + HW slope)
OPT_TRIM = True        # causal-trim sc/ya/lb boundary matmuls
OPT_LB_FOLD = True     # fold inner attnT tiles 4:1 on GpSimd before lb
OPT_SSQ_GP = True      # fused square+reduce on GpSimd (else DVE 2-op)
OPT_COPY_DVE = True    # move PSUM evacuation copies ACT -> DVE


def _build_program(loop_n=0):
    import concourse.mybir as mybir
    import concourse.tile as tile
    from concourse import bacc
    from concourse.masks import make_identity

    f32 = mybir.dt.float32
    bf16 = mybir.dt.bfloat16
    AL = mybir.AluOpType
    AF = mybir.ActivationFunctionType
    AX = mybir.AxisListType

    nc = bacc.Bacc("TRN2", target_bir_lowering=False, debug=False,
                   enable_asserts=True, num_devices=N_CORES)

    xT = nc.dram_tensor("xT", [D, S], bf16, kind="ExternalInput").ap()
    wcat = nc.dram_tensor("wcat", [D, QW + KVW], bf16,
                          kind="ExternalInput").ap()
    wp = nc.dram_tensor("wp", [4 * HD, D], bf16, kind="ExternalInput").ap()
    cosd = nc.dram_tensor("cosd", [S, HD // 2], bf16,
                          kind="ExternalInput").ap()
    sind = nc.dram_tensor("sind", [S, HD // 2], bf16,
                          kind="ExternalInput").ap()
    rmaskd = nc.dram_tensor("rmaskd", [P, P], f32, kind="ExternalInput").ap()
    gaind = nc.dram_tensor("gaind", [P, 6], f32, kind="ExternalInput").ap()
    outd = nc.dram_tensor("out", [S, D], bf16, kind="ExternalOutput").ap()

    xT_r = xT.rearrange("(kt p) t -> p kt t", p=P)        # [128, 16, S]
    wcat_r = wcat.rearrange("(kt p) n -> p kt n", p=P)    # [128, 16, 768]
    wp_r = wp.rearrange("(ct p) o -> p ct o", p=P)        # [128, 4, D]
    cos_r = cosd.rearrange("(t p) c -> p t c", p=P)       # [128, 16, 64]
    sin_r = sind.rearrange("(t p) c -> p t c", p=P)

    import contextlib as _ctxlib
    with tile.TileContext(nc) as tc, _ctxlib.ExitStack() as _es:
        pc = _es.enter_context(tc.tile_pool(name="const", bufs=1))
        pb = _es.enter_context(tc.tile_pool(name="batch", bufs=1))
        px = _es.enter_context(tc.tile_pool(name="xs", bufs=2))
        pw = _es.enter_context(tc.tile_pool(name="work", bufs=2))
        pat = _es.enter_context(tc.tile_pool(name="attn", bufs=2))
        po = _es.enter_context(tc.tile_pool(name="outp", bufs=3))
        prl = _es.enter_context(tc.tile_pool(name="rlp", bufs=2))
        prq = _es.enter_context(tc.tile_pool(name="rlq", bufs=2))
        pg = _es.enter_context(tc.tile_pool(name="gsum", bufs=3))
        # PSUM (8 banks): ppA = {pmm, sc} x 2 bufs = 4; ppB = {pmv, ptt,
        # ya, lb} x 1 buf = 4.
        ppA = _es.enter_context(tc.tile_pool(name="psA", bufs=2, space="PSUM"))
        ppB = _es.enter_context(tc.tile_pool(name="psB", bufs=1, space="PSUM"))

        # ---- constants resident in SBUF
        wcat_sb = pc.tile([P, TT_B, QW + KVW], bf16, tag="wcat")
        for kt in range(TT_B):
            nc.sync.dma_start(wcat_sb[:, kt, :], wcat_r[:, kt, :])
        wp_sb = pc.tile([P, 4, D], bf16, tag="wp")
        nc.sync.dma_start(wp_sb[:], wp_r[:])
        cos_sb = pc.tile([P, TT_B, HD // 2], bf16, tag="cos")
        nc.sync.dma_start(cos_sb[:], cos_r[:])
        sin_sb = pc.tile([P, TT_B, HD // 2], bf16, tag="sin")
        nc.sync.dma_start(sin_sb[:], sin_r[:])
        rmask_sb = pc.tile([P, P], f32, tag="rmask")
        nc.sync.dma_start(rmask_sb[:], rmaskd[:])
        gain_sb = pc.tile([P, 6], f32, tag="gain")
        nc.sync.dma_start(gain_sb[:], gaind[:])
        idf = pc.tile([P, P], f32, tag="idf")
        make_identity(nc, idf[:])
        idb = pc.tile([P, P], bf16, tag="idb")
        nc.vector.tensor_copy(idb[:], idf[:])
        ones_sb = pc.tile([P, P], bf16, tag="ones")
        nc.vector.memset(ones_sb[:], 1.0)

        for _ in range(max(1, loop_n)):
            # qkT packs [q0..q3, k] transposed: [128, 5, 1024] per half
            qkT_h = [pb.tile([P, NSEG, S // 2], bf16, tag="qkTlo",
                             name="qkTlo"),
                     pb.tile([P, NSEG, S // 2], bf16, tag="qkThi",
                             name="qkThi")]

            def qT_grp(hh, g, c0=0):
                half_i, loc = divmod(g * 4, TT_B // 2)
                return qkT_h[half_i][:, hh, loc * P + c0:(loc + 4) * P]

            def kT_at(jt):
                half_i, loc = divmod(jt, TT_B // 2)
                return qkT_h[half_i][:, 4, loc * P:(loc + 1) * P]

            def vN_at(jt):
                # v lives in the last segment of the per-tile staging buffer
                return stgs[jt][:, QW + HD:QW + 2 * HD]

            yT = pb.tile([P, 4, S], bf16, tag="yT")

            # ======= QKV projection + RMS + RoPE (software-pipelined) =====
            stgs = {}
            xts = {}
            ssqs = {}
            rsgs = {}

            def project_quarter(qq):
                hh = qq // 2
                if qq % 2 == 0:
                    ssqs[hh] = pb.tile([P, HB, NSEG], f32, tag=f"ssq{hh}",
                                       name=f"ssq{hh}")
                for tt in range(4 * qq, 4 * qq + 4):
                    t0 = tt * P
                    if tt % 4 == 0:
                        xt = px.tile([P, TT_B, 4 * P], bf16, tag="xt")
                        xts[qq] = xt
                        nc.sync.dma_start(xt[:], xT_r[:, :, t0:t0 + 4 * P])
                    xt = xts[qq]
                    xoff = (tt % 4) * P

                    pp = ppA.tile([P, QW], f32, tag="pmm", name="pmm")
                    for kt in range(TT_B):
                        nc.tensor.matmul(pp[:], xt[:, kt, xoff:xoff + P],
                                         wcat_sb[:, kt, :QW],
                                         start=(kt == 0), stop=(kt == TT_B - 1))
                    pv = ppB.tile([P, KVW], f32, tag="pmv", name="pmv")
                    for kt in range(TT_B):
                        nc.tensor.matmul(pv[:], xt[:, kt, xoff:xoff + P],
                                         wcat_sb[:, kt, QW:],
                                         start=(kt == 0), stop=(kt == TT_B - 1))

                    # stage q0..q3,k,v in SBUF (bf16): q via DVE, k+v in one
                    # ACT copy straight out of PSUM
                    stg = pb.tile([P, (NSEG + 1) * HD], bf16, tag=f"stg{tt}")
                    stgs[tt] = stg
                    if OPT_COPY_DVE:
                        nc.vector.tensor_copy(stg[:, :QW], pp[:])
                    else:
                        nc.scalar.copy(stg[:, :QW], pp[:])
                    nc.scalar.copy(stg[:, QW:QW + 2 * HD], pv[:])
                    stg_v = stg[:, :NSEG * HD].rearrange(
                        "p (s x) -> p s x", s=NSEG)
                    if OPT_SSQ_GP:
                        # square on GpSimd (idle engine), per-seg reduce on DVE
                        scr = prq.tile([P, NSEG * HD], f32, tag="scr")
                        nc.gpsimd.tensor_tensor(scr[:], stg[:, :NSEG * HD],
                                                stg[:, :NSEG * HD], AL.mult)
                        nc.vector.tensor_reduce(
                            ssqs[hh][:, tt % HB, :],
                            scr[:].rearrange("p (s x) -> p s x", s=NSEG),
                            axis=AX.X, op=AL.add)
                    else:
                        scr = prq.tile([P, NSEG * HD], f32, tag="scr")
                        nc.vector.tensor_tensor(scr[:], stg[:, :NSEG * HD],
                                                stg[:, :NSEG * HD], AL.mult)
                        nc.vector.tensor_reduce(
                            ssqs[hh][:, tt % HB, :],
                            scr[:].rearrange("p (s x) -> p s x", s=NSEG),
                            axis=AX.X, op=AL.add)

            def rsqrt_half(hh):
                # rs = exp(-0.5*ln(ssq/HD+eps)) * gain
                lnb = pb.tile([P, HB, NSEG], f32, tag=f"lnb{hh}")
                nc.scalar.activation(lnb[:], ssqs[hh][:], AF.Ln,
                                     scale=1.0 / HD, bias=gain_sb[:, 5:6])
                rsb = pb.tile([P, HB, NSEG], f32, tag=f"rsb{hh}")
                nc.scalar.activation(rsb[:], lnb[:], AF.Exp, scale=-0.5)
                rsg = pb.tile([P, HB, NSEG], f32, tag=f"rsg{hh}")
                nc.vector.tensor_tensor(
                    rsg[:], rsb[:],
                    gain_sb[:, None, :NSEG].to_broadcast([P, HB, NSEG]),
                    AL.mult)
                rsgs[hh] = rsg

            def rope_quarter(qq):
                rsg = rsgs[qq // 2]
                for tt in range(4 * qq, 4 * qq + 4):
                    ppv = stgs[tt][:, :NSEG * HD].rearrange(
                        "p (s x) -> p s x", s=NSEG)
                    qn = pw.tile([P, NSEG, HD], bf16, tag="qn")
                    nc.vector.tensor_tensor(
                        qn[:], ppv,
                        rsg[:, tt % HB, :, None].to_broadcast([P, NSEG, HD]),
                        AL.mult)

                    # rope: out1 = a*cos + b2*sin ; out2 = b2*cos - a*sin
                    a = qn[:, :, :HD // 2]
                    b2 = qn[:, :, HD // 2:]
                    rpb = pw.tile([P, NSEG, HD], bf16, tag="rpb")
                    o1 = rpb[:, :, :HD // 2]
                    o2 = rpb[:, :, HD // 2:]
                    t1 = pw.tile([P, NSEG, HD // 2], bf16, tag="t1")
                    t2 = pw.tile([P, NSEG, HD // 2], bf16, tag="t2")
                    cb = cos_sb[:, tt:tt + 1, :].to_broadcast(
                        [P, NSEG, HD // 2])
                    sb_ = sin_sb[:, tt:tt + 1, :].to_broadcast(
                        [P, NSEG, HD // 2])
                    nc.gpsimd.tensor_tensor(t1[:], a, cb, AL.mult)
                    nc.vector.tensor_tensor(t2[:], b2, sb_, AL.mult)
                    nc.gpsimd.tensor_tensor(o1, t1[:], t2[:], AL.add)
                    nc.gpsimd.tensor_tensor(t1[:], b2, cb, AL.mult)
                    nc.vector.tensor_tensor(t2[:], a, sb_, AL.mult)
                    nc.vector.tensor_tensor(o2, t1[:], t2[:], AL.subtract)

                    # transpose q0..q3,k into [head_dim, token] (bf16)
                    rpf = rpb[:].rearrange("p s x -> p (s x)")
                    ptq = ppB.tile([P, NSEG * P], bf16, tag="ptt", name="ptt")
                    for sseg in range(NSEG):
                        nc.tensor.transpose(ptq[:, sseg * P:(sseg + 1) * P],
                                            rpf[:, sseg * P:(sseg + 1) * P],
                                            idb[:])
                    half_i, loc = divmod(tt, TT_B // 2)
                    nc.vector.tensor_copy(
                        qkT_h[half_i][:, :, loc * P:(loc + 1) * P],
                        ptq[:].rearrange("p (s x) -> p s x", s=NSEG))

            # ================= attention (+ interleaved out-proj) =========
            def proj_block(tt_list):
                for tt in tt_list:
                    ob = po.tile([P, D], bf16, tag="ob")
                    for oc in range(4):
                        pout = ppA.tile([P, 512], f32, tag="pmm", name="pout")
                        for ct in range(4):
                            nc.tensor.matmul(
                                pout[:], yT[:, ct, tt * P:(tt + 1) * P],
                                wp_sb[:, ct, oc * 512:(oc + 1) * 512],
                                start=(ct == 0), stop=(ct == 3))
                        nc.vector.tensor_copy(
                            ob[:, oc * 512:(oc + 1) * 512], pout[:])
                    nc.sync.dma_start(
                        outd[tt * P:(tt + 1) * P, :], ob[:])

            def attn_group(g):
                nj = 4 * (g + 1)       # key tiles for this group
                for h in range(4):
                    attnT = pat.tile([P, TT_B, 512], bf16, tag="attnT")
                    # l replicated across 128 partitions: same matmul cost,
                    # full-width reciprocal, no partition broadcast.
                    lb = ppB.tile([P, 512], f32, tag="lb", name="lb")
                    ya = ppB.tile([P, 512], f32, tag="ya", name="ya")

                    # lb accumulates g pre-folded inner sums (4 attnT tiles
                    # summed on GpSimd) + 4 trimmed boundary tiles.
                    n_lb = (g + 4) if OPT_LB_FOLD else nj
                    lb_n = [0]

                    def lb_emit(src_ap, c0):
                        lb_n[0] += 1
                        nc.tensor.matmul(lb[:, c0:], ones_sb[:], src_ap,
                                         start=(lb_n[0] == 1),
                                         stop=(lb_n[0] == n_lb))

                    def ya_emit(jt, c0, stop):
                        nc.tensor.matmul(ya[:, c0:], vN_at(jt),
                                         attnT[:, jt, c0:],
                                         start=(jt == 0), stop=stop)

                    pairs = {}
                    Ss = {}
                    lb_pend = []   # (ready_step, emit_fn), FIFO
                    prev = None
                    for jt in range(nj):
                        jj = jt - 4 * g
                        inner = jj < 0
                        c0 = jj * 128 if jj > 0 else 0
                        sc = ppA.tile([P, 512], f32, tag="sc", name="sc")
                        # causal trim: only columns >= c0 are live
                        nc.tensor.matmul(sc[:, c0:], kT_at(jt),
                                         qT_grp(h, g, c0),
                                         start=True, stop=True)
                        if jj >= 0:
                            # triangular mask on the boundary block
                            nc.vector.tensor_tensor(
                                sc[:, c0:c0 + 128], sc[:, c0:c0 + 128],
                                rmask_sb[:], AL.add)
                        nc.scalar.activation(attnT[:, jt, c0:], sc[:, c0:],
                                             AF.Exp)
                        if inner and OPT_LB_FOLD:
                            if jt % 2 == 1:
                                pr = pg.tile([P, 512], bf16, tag="spair")
                                nc.gpsimd.tensor_tensor(
                                    pr[:], attnT[:, jt - 1, :],
                                    attnT[:, jt, :], AL.add)
                                pairs[jt // 2] = pr
                            if jt % 4 == 3:
                                k4 = jt // 4
                                Sg = pg.tile([P, 512], bf16, tag="sgrp")
                                nc.gpsimd.tensor_tensor(
                                    Sg[:], pairs[2 * k4][:],
                                    pairs[2 * k4 + 1][:], AL.add)
                                Ss[k4] = Sg
                                lb_pend.append(
                                    (jt + 2,
                                     lambda k4=k4: lb_emit(Ss[k4][:], 0)))
                        elif inner:
                            lb_pend.append(
                                (jt + 1,
                                 lambda jt=jt: lb_emit(attnT[:, jt, :], 0)))
                        else:
                            lb_pend.append(
                                (jt + 1,
                                 lambda jt=jt, c0=c0: lb_emit(
                                     attnT[:, jt, c0:], c0)))
                        if prev is not None:
                            ya_emit(*prev, stop=False)
                        while lb_pend and lb_pend[0][0] <= jt:
                            lb_pend.pop(0)[1]()
                        prev = (jt, c0)
                    ya_emit(*prev, stop=True)
                    for _, fn in lb_pend:
                        fn()

                    rlb = prl.tile([P, 512], f32, tag="rlb")
                    nc.vector.reciprocal_approx_fast(rlb[:], lb[:])
                    nc.vector.tensor_tensor(
                        yT[:, h, g * 512:(g + 1) * 512], ya[:], rlb[:],
                        AL.mult)

                    # out-proj for the previous group's tokens, interleaved
                    # one token tile per head so the PE stream stays dense.
                    if g >= 1:
                        proj_block([4 * (g - 1) + h])

            project_quarter(0)
            project_quarter(1)
            rsqrt_half(0)
            project_quarter(2)
            rope_quarter(0)
            project_quarter(3)
            rope_quarter(1)
            rsqrt_half(1)
            attn_group(0)
            rope_quarter(2)
            attn_group(1)
            rope_quarter(3)
            attn_group(2)
            attn_group(3)
            proj_block(range(4 * (GROUPS - 1), 4 * GROUPS))

    nc.compile()
    return nc


def _get_program(loop_n=0):
    key = loop_n
    if key not in _PROG:
        _PROG[key] = _build_program(loop_n)
    return _PROG[key]


def _host_prep(x, Wq, Wk, Wv, Wp, q_gain):
    """Build the 8 per-core input maps: core c -> (kv head c//2, batch c%2)."""
    import ml_dtypes
    bf16 = ml_dtypes.bfloat16

    x = np.ascontiguousarray(x.reshape(B, S, D), dtype=np.float32)
    xTb = [np.ascontiguousarray(x[b].T.astype(bf16)) for b in range(B)]

    inv_freq = 1.0 / (ROPE_BASE ** (np.arange(0, HD, 2, dtype=np.float32) / HD))
    freqs = np.arange(S, dtype=np.float32)[:, None] * inv_freq[None, :]
    cos = np.ascontiguousarray(np.cos(freqs).astype(bf16))   # [S, 64]
    sin = np.ascontiguousarray(np.sin(freqs).astype(bf16))

    r = np.arange(P)[:, None]
    c = np.arange(P)[None, :]
    rmask = np.where(c < r, NEG, 0.0).astype(np.float32)   # [128, 128] tri

    scale = 1.0 / math.sqrt(HD)
    in_maps = []
    for core in range(N_CORES):
        kv = core // 2
        b = core % 2
        h0 = 4 * kv
        WqT = Wq[h0 * HD:(h0 + 4) * HD, :].T             # [D, 512]
        WkT = Wk[kv * HD:(kv + 1) * HD, :].T             # [D, 128]
        WvT = Wv[kv * HD:(kv + 1) * HD, :].T             # [D, 128]
        wcat = np.ascontiguousarray(
            np.concatenate([WqT, WkT, WvT], axis=1).astype(bf16))
        wpT = np.ascontiguousarray(
            Wp[:, h0 * HD:(h0 + 4) * HD].T.astype(bf16))        # [512, D]
        gain = np.tile(np.array(
            [[q_gain[h0] * scale, q_gain[h0 + 1] * scale,
              q_gain[h0 + 2] * scale, q_gain[h0 + 3] * scale, 1.0, EPS]],
            dtype=np.float32), (P, 1))
        in_maps.append({
            "xT": xTb[b],
            "wcat": wcat,
            "wp": wpT,
            "cosd": cos,
            "sind": sin,
            "rmaskd": rmask,
            "gaind": np.ascontiguousarray(gain),
        })
    return in_maps


def kernel(x, Wq, Wk, Wv, Wp, q_gain):
    from concourse.bass_utils import run_bass_kernel_spmd

    nc = _get_program()
    in_maps = _host_prep(x, Wq, Wk, Wv, Wp, q_gain)
    try:
        res = run_bass_kernel_spmd(nc, in_maps, core_ids=list(range(N_CORES)))
    except Exception:
        # one retry: a previous crashed run can leave the exec unit wedged
        res = run_bass_kernel_spmd(nc, in_maps, core_ids=list(range(N_CORES)))
    total = np.zeros((B, S, D), dtype=np.float32)
    for core, r in enumerate(res.results):
        total[core % 2] += r["out"].astype(np.float32)
    return total



# revision 37
# speedup vs baseline: 1.1805x; 1.1805x over previous
"""Causal GQA self-attention (B=2, S=2048, D=2048, 16 heads / 4 KV heads) on 8
Trainium2 NeuronCores.

Sharding: (kv-head x batch). Core c owns kv head c//2 and batch c%2: it
projects the 4 GQA query heads of that kv head plus k,v for its batch's 2048
tokens (no duplicated k/v work), runs causal attention for those 4 heads, and
computes the partial output projection for its batch's tokens. The host sums
the 4 kv-group partials per batch.

Device-side design (carried over from the head-TP version):
  - All matmuls bf16 (PSUM f32); x/weights/outputs cross DMA as bf16.
  - Scores are computed transposed: sc[key, query] = kT_tile^T @ qT_group,
    feeding AV directly with no PE transposes. The softmax denominator comes
    from a ones[128,128]-stationary matmul -> lb[128,512] (128 replicated
    rows) -> single full-width reciprocal_approx_fast.
  - QKV projection runs in two passes per token tile (q: 512 wide, kv: 256
    wide) to respect PSUM bank limits; RMS+RoPE is software-pipelined so the
    PE queue never waits on the DVE/Pool rope chain.
"""

import math

import numpy as np

B = 2
S = 2048
D = 2048
T = B * S
NH = 16
NKV = 4
HD = 128
P = 128
ROPE_BASE = 10000.0
EPS = float(np.finfo(np.float32).eps)
NEG = -1.0e30

N_CORES = 8
TT_B = S // P          # 16 token tiles per batch
GROUPS = 4             # groups of 4 q-tiles (512 queries)
QW = 512               # q projection width (4 heads)
KVW = 256              # k+v projection width
NSEG = 5               # rms/rope segments: q0..q3, k
HB = 8                 # token tiles per rsqrt batch

_PROG = {}

import os


def _env(name, default):
    v = os.environ.get(name)
    return default if v is None else bool(int(v))


# tuning flags (A/B'd via TimelineSim + HW slope)
OPT_TRIM = True        # causal-trim sc/ya/lb boundary matmuls
OPT_LB_FOLD = _env("KOPT_LB_FOLD", False)
OPT_SSQ_GP = _env("KOPT_SSQ_GP", False)
# NOTE: tensor_tensor_reduce (fused square+reduce) crashes real HW
# (NRT_EXEC_UNIT_UNRECOVERABLE) though CoreSim accepts it -- keep off.
OPT_SSQ_FUSED = _env("KOPT_SSQ_FUSED", False)
OPT_COPY_DVE = _env("KOPT_COPY_DVE", True)
OPT_SPREAD = _env("KOPT_SPREAD", True)
OPT_QN_GP = _env("KOPT_QN_GP", False)
OPT_ROPE_GP = _env("KOPT_ROPE_GP", False)
OPT_MASK_PE = _env("KOPT_MASK_PE", True)
OPT_O2_GP = _env("KOPT_O2_GP", False)
OPT_RSQRT_DVE = _env("KOPT_RSQRT_DVE", True)
OPT_OB_DVE = int(os.environ.get("KOPT_OB_DVE", "2"))


def _build_program(loop_n=0):
    import concourse.mybir as mybir
    import concourse.tile as tile
    from concourse import bacc
    from concourse.masks import make_identity

    f32 = mybir.dt.float32
    bf16 = mybir.dt.bfloat16
    AL = mybir.AluOpType
    AF = mybir.ActivationFunctionType
    AX = mybir.AxisListType

    nc = bacc.Bacc("TRN2", target_bir_lowering=False, debug=False,
                   enable_asserts=True, num_devices=N_CORES)

    xT = nc.dram_tensor("xT", [D, S], bf16, kind="ExternalInput").ap()
    wcat = nc.dram_tensor("wcat", [D, QW + KVW], bf16,
                          kind="ExternalInput").ap()
    wp = nc.dram_tensor("wp", [4 * HD, D], bf16, kind="ExternalInput").ap()
    cosd = nc.dram_tensor("cosd", [S, HD // 2], bf16,
                          kind="ExternalInput").ap()
    sind = nc.dram_tensor("sind", [S, HD // 2], bf16,
                          kind="ExternalInput").ap()
    rmaskd = nc.dram_tensor("rmaskd", [P, P], f32, kind="ExternalInput").ap()
    gaind = nc.dram_tensor("gaind", [P, 6], f32, kind="ExternalInput").ap()
    outd = nc.dram_tensor("out", [S, D], bf16, kind="ExternalOutput").ap()

    xT_r = xT.rearrange("(kt p) t -> p kt t", p=P)        # [128, 16, S]
    wcat_r = wcat.rearrange("(kt p) n -> p kt n", p=P)    # [128, 16, 768]
    wp_r = wp.rearrange("(ct p) o -> p ct o", p=P)        # [128, 4, D]
    cos_r = cosd.rearrange("(t p) c -> p t c", p=P)       # [128, 16, 64]
    sin_r = sind.rearrange("(t p) c -> p t c", p=P)

    import contextlib as _ctxlib
    with tile.TileContext(nc) as tc, _ctxlib.ExitStack() as _es:
        pc = _es.enter_context(tc.tile_pool(name="const", bufs=1))
        pb = _es.enter_context(tc.tile_pool(name="batch", bufs=1))
        px = _es.enter_context(tc.tile_pool(name="xs", bufs=2))
        pw = _es.enter_context(tc.tile_pool(name="work", bufs=2))
        pat = _es.enter_context(tc.tile_pool(name="attn", bufs=2))
        po = _es.enter_context(tc.tile_pool(name="outp", bufs=3))
        prl = _es.enter_context(tc.tile_pool(name="rlp", bufs=2))
        prq = _es.enter_context(tc.tile_pool(name="rlq", bufs=2))
        pg = _es.enter_context(tc.tile_pool(name="gsum", bufs=3))
        # PSUM (8 banks): ppA = {pmm, sc} x 2 bufs = 4; ppB = {pmv, ptt,
        # ya, lb} x 1 buf = 4.
        ppA = _es.enter_context(tc.tile_pool(name="psA", bufs=2, space="PSUM"))
        ppB = _es.enter_context(tc.tile_pool(name="psB", bufs=1, space="PSUM"))

        # ---- constants resident in SBUF.  DMA order matters for the NEFF
        # startup: the first projection matmul needs x tile 0 (2MB) and
        # wcat[kt=0]; wp isn't read until the first out-proj ~halfway in.
        xt0 = px.tile([P, TT_B, 4 * P], bf16, tag="xt")
        nc.sync.dma_start(xt0[:], xT_r[:, :, 0:4 * P])
        wcat_sb = pc.tile([P, TT_B, QW + KVW], bf16, tag="wcat")
        for kt in range(TT_B):
            nc.sync.dma_start(wcat_sb[:, kt, :], wcat_r[:, kt, :])
        gain_sb = pc.tile([P, 6], f32, tag="gain")
        nc.sync.dma_start(gain_sb[:], gaind[:])
        cos_sb = pc.tile([P, TT_B, HD // 2], bf16, tag="cos")
        nc.sync.dma_start(cos_sb[:], cos_r[:])
        sin_sb = pc.tile([P, TT_B, HD // 2], bf16, tag="sin")
        nc.sync.dma_start(sin_sb[:], sin_r[:])
        rmask_sb = pc.tile([P, P], f32, tag="rmask")
        nc.sync.dma_start(rmask_sb[:], rmaskd[:])
        wp_sb = pc.tile([P, 4, D], bf16, tag="wp")
        nc.sync.dma_start(wp_sb[:], wp_r[:])
        idf = pc.tile([P, P], f32, tag="idf")
        make_identity(nc, idf[:])
        idb = pc.tile([P, P], bf16, tag="idb")
        nc.vector.tensor_copy(idb[:], idf[:])
        ones_sb = pc.tile([P, P], bf16, tag="ones")
        nc.vector.memset(ones_sb[:], 1.0)
        rmb = pc.tile([P, P], bf16, tag="rmb")
        nc.vector.tensor_copy(rmb[:], rmask_sb[:])

        for body_i in range(max(1, loop_n)):
            # qkT packs [q0..q3, k] transposed: [128, 5, 1024] per half
            qkT_h = [pb.tile([P, NSEG, S // 2], bf16, tag="qkTlo",
                             name="qkTlo"),
                     pb.tile([P, NSEG, S // 2], bf16, tag="qkThi",
                             name="qkThi")]

            def qT_grp(hh, g, c0=0):
                half_i, loc = divmod(g * 4, TT_B // 2)
                return qkT_h[half_i][:, hh, loc * P + c0:(loc + 4) * P]

            def kT_at(jt):
                half_i, loc = divmod(jt, TT_B // 2)
                return qkT_h[half_i][:, 4, loc * P:(loc + 1) * P]

            def vN_at(jt):
                # v lives in the last segment of the per-tile staging buffer
                return stgs[jt][:, QW + HD:QW + 2 * HD]

            yT = pb.tile([P, 4, S], bf16, tag="yT")

            # ======= QKV projection + RMS + RoPE (software-pipelined) =====
            stgs = {}
            xts = {}
            ssqs = {}
            rsgs = {}

            def project_tile(tt):
                qq = tt // 4
                t0 = tt * P
                if tt % 4 == 0:
                    if qq == 0 and body_i == 0:
                        xts[0] = xt0   # prefetched before the constants
                    else:
                        xt = px.tile([P, TT_B, 4 * P], bf16, tag="xt")
                        xts[qq] = xt
                        nc.sync.dma_start(xt[:], xT_r[:, :, t0:t0 + 4 * P])
                    ssqs[qq] = pb.tile([P, 4, NSEG], f32, tag=f"ssq{qq}",
                                       name=f"ssq{qq}")
                xt = xts[qq]
                xoff = (tt % 4) * P

                pp = ppA.tile([P, QW], f32, tag="pmm", name="pmm")
                for kt in range(TT_B):
                    nc.tensor.matmul(pp[:], xt[:, kt, xoff:xoff + P],
                                     wcat_sb[:, kt, :QW],
                                     start=(kt == 0), stop=(kt == TT_B - 1))
                pv = ppB.tile([P, KVW], f32, tag="pmv", name="pmv")
                for kt in range(TT_B):
                    nc.tensor.matmul(pv[:], xt[:, kt, xoff:xoff + P],
                                     wcat_sb[:, kt, QW:],
                                     start=(kt == 0), stop=(kt == TT_B - 1))

                # stage q0..q3,k,v in SBUF (bf16): q via DVE, k+v in one
                # ACT copy straight out of PSUM
                stg = pb.tile([P, (NSEG + 1) * HD], bf16, tag=f"stg{tt}")
                stgs[tt] = stg
                if OPT_COPY_DVE:
                    nc.vector.tensor_copy(stg[:, :QW], pp[:])
                else:
                    nc.scalar.copy(stg[:, :QW], pp[:])
                nc.scalar.copy(stg[:, QW:QW + 2 * HD], pv[:])
                stg_v = stg[:, :NSEG * HD].rearrange("p (s x) -> p s x",
                                                     s=NSEG)
                if OPT_SSQ_GP:
                    # square on GpSimd (idle engine), per-seg reduce on DVE
                    scr = prq.tile([P, NSEG * HD], f32, tag="scr")
                    nc.gpsimd.tensor_tensor(scr[:], stg[:, :NSEG * HD],
                                            stg[:, :NSEG * HD], AL.mult)
                    nc.vector.tensor_reduce(
                        ssqs[qq][:, tt % 4, :],
                        scr[:].rearrange("p (s x) -> p s x", s=NSEG),
                        axis=AX.X, op=AL.add)
                elif OPT_SSQ_FUSED:
                    # one fused square+reduce DVE op per segment
                    scr = prq.tile([P, NSEG, HD], f32, tag="scr")
                    for sseg in range(NSEG):
                        nc.vector.tensor_tensor_reduce(
                            scr[:, sseg, :], stg_v[:, sseg, :],
                            stg_v[:, sseg, :], 1.0, 0.0,
                            AL.mult, AL.add,
                            accum_out=ssqs[qq][:, tt % 4, sseg:sseg + 1])
                else:
                    scr = prq.tile([P, NSEG * HD], f32, tag="scr")
                    nc.vector.tensor_tensor(scr[:], stg[:, :NSEG * HD],
                                            stg[:, :NSEG * HD], AL.mult)
                    nc.vector.tensor_reduce(
                        ssqs[qq][:, tt % 4, :],
                        scr[:].rearrange("p (s x) -> p s x", s=NSEG),
                        axis=AX.X, op=AL.add)

            def rsqrt_quarter(qq):
                if not OPT_RSQRT_DVE:
                    # rs = exp(-0.5*ln(ssq/HD+eps)) * gain on ACT
                    lnb = pb.tile([P, 4, NSEG], f32, tag=f"lnb{qq}")
                    nc.scalar.activation(lnb[:], ssqs[qq][:], AF.Ln,
                                         scale=1.0 / HD, bias=gain_sb[:, 5:6])
                    rsb = pb.tile([P, 4, NSEG], f32, tag=f"rsb{qq}")
                    nc.scalar.activation(rsb[:], lnb[:], AF.Exp, scale=-0.5)
                    rsg = pb.tile([P, 4, NSEG], f32, tag=f"rsg{qq}")
                    nc.vector.tensor_tensor(
                        rsg[:], rsb[:],
                        gain_sb[:, None, :NSEG].to_broadcast([P, 4, NSEG]),
                        AL.mult)
                    rsgs[qq] = rsg
                    return
                # rs = rsqrt(ssq/HD + eps) * gain, computed entirely on DVE
                # (magic-constant seed + 2 Newton steps) so no ACT table
                # switches (Ln/Exp live in different act-func sets than the
                # softmax Exp and each switch costs ~1.3us).
                i32 = mybir.dt.int32
                m = pb.tile([P, 4, NSEG], f32, tag=f"lnb{qq}")
                nc.vector.tensor_scalar(m[:], ssqs[qq][:], 1.0 / HD, EPS,
                                        AL.mult, AL.add)
                ya_ = pb.tile([P, 4, NSEG], f32, tag=f"rsb{qq}")
                yb_ = pb.tile([P, 4, NSEG], f32, tag=f"rsc{qq}")
                # y0 = bitcast(0x5f3759df - (bitcast(m) >> 1))
                #    = bitcast((~(bitcast(m) >> 1)) + 0x5f3759e0)
                nc.vector.tensor_scalar(yb_[:].bitcast(i32), m[:].bitcast(i32),
                                        1, -1,
                                        AL.arith_shift_right, AL.bitwise_xor)
                nc.vector.tensor_scalar(ya_[:].bitcast(i32),
                                        yb_[:].bitcast(i32),
                                        0x5f3759e0, None, AL.add)
                # two Newton iterations: y <- y * (1.5 - 0.5*m*y*y)
                for ysrc, ydst in ((ya_, yb_), (yb_, ya_)):
                    t = pb.tile([P, 4, NSEG], f32, tag=f"rst{qq}")
                    nc.vector.tensor_tensor(t[:], ysrc[:], ysrc[:], AL.mult)
                    nc.vector.scalar_tensor_tensor(t[:], t[:], -0.5, m[:],
                                                   AL.mult, AL.mult)
                    nc.vector.scalar_tensor_tensor(ydst[:], t[:], 1.5,
                                                   ysrc[:], AL.add, AL.mult)
                rsg = pb.tile([P, 4, NSEG], f32, tag=f"rsg{qq}")
                nc.vector.tensor_tensor(
                    rsg[:], ya_[:],
                    gain_sb[:, None, :NSEG].to_broadcast([P, 4, NSEG]),
                    AL.mult)
                rsgs[qq] = rsg

            def rope_tile(tt, qkt_dve=True):
                rsg = rsgs[tt // 4]
                ppv = stgs[tt][:, :NSEG * HD].rearrange("p (s x) -> p s x",
                                                        s=NSEG)
                qn = pw.tile([P, NSEG, HD], bf16, tag="qn")
                rs_b = rsg[:, tt % 4, :, None].to_broadcast([P, NSEG, HD])
                if OPT_QN_GP:
                    nc.gpsimd.tensor_tensor(qn[:], ppv, rs_b, AL.mult)
                else:
                    nc.vector.tensor_tensor(qn[:], ppv, rs_b, AL.mult)

                # rope: out1 = a*cos + b2*sin ; out2 = b2*cos - a*sin
                a = qn[:, :, :HD // 2]
                b2 = qn[:, :, HD // 2:]
                rpb = pw.tile([P, NSEG, HD], bf16, tag="rpb")
                o1 = rpb[:, :, :HD // 2]
                o2 = rpb[:, :, HD // 2:]
                t1 = pw.tile([P, NSEG, HD // 2], bf16, tag="t1")
                t2 = pw.tile([P, NSEG, HD // 2], bf16, tag="t2")
                cb = cos_sb[:, tt:tt + 1, :].to_broadcast([P, NSEG, HD // 2])
                sb_ = sin_sb[:, tt:tt + 1, :].to_broadcast([P, NSEG, HD // 2])
                eng1 = nc.gpsimd if OPT_ROPE_GP else nc.vector
                eng1.tensor_tensor(t1[:], a, cb, AL.mult)
                nc.vector.tensor_tensor(t2[:], b2, sb_, AL.mult)
                eng1.tensor_tensor(o1, t1[:], t2[:], AL.add)
                eng1.tensor_tensor(t1[:], b2, cb, AL.mult)
                nc.vector.tensor_tensor(t2[:], a, sb_, AL.mult)
                if OPT_O2_GP:
                    nc.gpsimd.tensor_tensor(o2, t1[:], t2[:], AL.subtract)
                else:
                    nc.vector.tensor_tensor(o2, t1[:], t2[:], AL.subtract)

                # transpose q0..q3,k into [head_dim, token] (bf16)
                rpf = rpb[:].rearrange("p s x -> p (s x)")
                ptq = ppB.tile([P, NSEG * P], bf16, tag="ptt", name="ptt")
                for sseg in range(NSEG):
                    nc.tensor.transpose(ptq[:, sseg * P:(sseg + 1) * P],
                                        rpf[:, sseg * P:(sseg + 1) * P],
                                        idb[:])
                half_i, loc = divmod(tt, TT_B // 2)
                dst = qkT_h[half_i][:, :, loc * P:(loc + 1) * P]
                src = ptq[:].rearrange("p (s x) -> p s x", s=NSEG)
                if qkt_dve:
                    nc.vector.tensor_copy(dst, src)
                else:
                    nc.scalar.copy(dst, src)

            # ================= attention (+ interleaved out-proj) =========
            def proj_block(tt_list):
                for tt in tt_list:
                    ob = po.tile([P, D], bf16, tag="ob")
                    for oc in range(4):
                        pout = ppA.tile([P, 512], f32, tag="pmm", name="pout")
                        for ct in range(4):
                            nc.tensor.matmul(
                                pout[:], yT[:, ct, tt * P:(tt + 1) * P],
                                wp_sb[:, ct, oc * 512:(oc + 1) * 512],
                                start=(ct == 0), stop=(ct == 3))
                        if oc < OPT_OB_DVE:
                            nc.vector.tensor_copy(
                                ob[:, oc * 512:(oc + 1) * 512], pout[:])
                        else:
                            nc.scalar.copy(ob[:, oc * 512:(oc + 1) * 512],
                                           pout[:])
                    nc.sync.dma_start(
                        outd[tt * P:(tt + 1) * P, :], ob[:])

            def attn_group(g, extra=None):
                nj = 4 * (g + 1)       # key tiles for this group
                for h in range(4):
                    attnT = pat.tile([P, TT_B, 512], bf16, tag="attnT")
                    # l replicated across 128 partitions: same matmul cost,
                    # full-width reciprocal, no partition broadcast.
                    lb = ppB.tile([P, 512], f32, tag="lb", name="lb")
                    ya = ppB.tile([P, 512], f32, tag="ya", name="ya")

                    # lb accumulates g pre-folded inner sums (4 attnT tiles
                    # summed on GpSimd) + 4 trimmed boundary tiles.
                    n_lb = (g + 4) if OPT_LB_FOLD else nj
                    lb_n = [0]

                    def lb_emit(src_ap, c0):
                        lb_n[0] += 1
                        nc.tensor.matmul(lb[:, c0:], ones_sb[:], src_ap,
                                         start=(lb_n[0] == 1),
                                         stop=(lb_n[0] == n_lb))

                    def ya_emit(jt, c0, stop):
                        nc.tensor.matmul(ya[:, c0:], vN_at(jt),
                                         attnT[:, jt, c0:],
                                         start=(jt == 0), stop=stop)

                    pairs = {}
                    Ss = {}
                    lb_pend = []   # (ready_step, emit_fn), FIFO
                    prev = None
                    for jt in range(nj):
                        jj = jt - 4 * g
                        inner = jj < 0
                        c0 = jj * 128 if jj > 0 else 0
                        sc = ppA.tile([P, 512], f32, tag="sc", name="sc")
                        # causal trim: only columns >= c0 are live
                        nc.tensor.matmul(sc[:, c0:], kT_at(jt),
                                         qT_grp(h, g, c0),
                                         start=True,
                                         stop=not (jj >= 0 and OPT_MASK_PE))
                        if jj >= 0:
                            # triangular mask on the boundary block
                            if OPT_MASK_PE:
                                nc.tensor.matmul(sc[:, c0:c0 + 128], idb[:],
                                                 rmb[:], start=False,
                                                 stop=True)
                            else:
                                nc.vector.tensor_tensor(
                                    sc[:, c0:c0 + 128], sc[:, c0:c0 + 128],
                                    rmask_sb[:], AL.add)
                        nc.scalar.activation(attnT[:, jt, c0:], sc[:, c0:],
                                             AF.Exp)
                        if inner and OPT_LB_FOLD:
                            if jt % 2 == 1:
                                pr = pg.tile([P, 512], bf16, tag="spair")
                                nc.gpsimd.tensor_tensor(
                                    pr[:], attnT[:, jt - 1, :],
                                    attnT[:, jt, :], AL.add)
                                pairs[jt // 2] = pr
                            if jt % 4 == 3:
                                k4 = jt // 4
                                Sg = pg.tile([P, 512], bf16, tag="sgrp")
                                nc.gpsimd.tensor_tensor(
                                    Sg[:], pairs[2 * k4][:],
                                    pairs[2 * k4 + 1][:], AL.add)
                                Ss[k4] = Sg
                                lb_pend.append(
                                    (jt + 2,
                                     lambda k4=k4: lb_emit(Ss[k4][:], 0)))
                        elif inner:
                            lb_pend.append(
                                (jt + 1,
                                 lambda jt=jt: lb_emit(attnT[:, jt, :], 0)))
                        else:
                            lb_pend.append(
                                (jt + 1,
                                 lambda jt=jt, c0=c0: lb_emit(
                                     attnT[:, jt, c0:], c0)))
                        if prev is not None:
                            ya_emit(*prev, stop=False)
                        while lb_pend and lb_pend[0][0] <= jt:
                            lb_pend.pop(0)[1]()
                        prev = (jt, c0)
                    ya_emit(*prev, stop=True)
                    for _, fn in lb_pend:
                        fn()

                    rlb = prl.tile([P, 512], f32, tag="rlb")
                    nc.vector.reciprocal_approx_fast(rlb[:], lb[:])
                    nc.vector.tensor_tensor(
                        yT[:, h, g * 512:(g + 1) * 512], ya[:], rlb[:],
                        AL.mult)

                    if extra is not None:
                        extra(h)

            if OPT_SPREAD:
                # software-pipelined: projection/rope tiles ride inside the
                # attention groups so PE-heavy and DVE/ACT-heavy work overlap
                for t in range(4):
                    project_tile(t)
                rsqrt_quarter(0)
                for t in range(4):
                    project_tile(4 + t)
                    rope_tile(t, qkt_dve=True)
                rsqrt_quarter(1)

                def ex0(h):
                    rope_tile(4 + h, qkt_dve=False)
                    project_tile(8 + h)
                    if h == 3:
                        rsqrt_quarter(2)

                def ex1(h):
                    rope_tile(8 + h, qkt_dve=False)
                    project_tile(12 + h)
                    if h == 3:
                        rsqrt_quarter(3)

                def ex2(h):
                    rope_tile(12 + h, qkt_dve=True)
                    proj_block([h])

                def ex3(h):
                    proj_block([4 + h])
                    proj_block([8 + h])

                attn_group(0, ex0)
                attn_group(1, ex1)
                attn_group(2, ex2)
                attn_group(3, ex3)
                proj_block(range(12, 16))
            else:
                for t in range(8):
                    project_tile(t)
                rsqrt_quarter(0)
                rsqrt_quarter(1)
                for t in range(4):
                    project_tile(8 + t)
                for t in range(4):
                    rope_tile(t)
                for t in range(4):
                    project_tile(12 + t)
                for t in range(4):
                    rope_tile(4 + t)
                rsqrt_quarter(2)
                rsqrt_quarter(3)
                attn_group(0, lambda h: None)
                for t in range(4):
                    rope_tile(8 + t)
                attn_group(1, lambda h: proj_block([h]))
                for t in range(4):
                    rope_tile(12 + t)
                attn_group(2, lambda h: proj_block([4 + h]))
                attn_group(3, lambda h: proj_block([8 + h]))
                proj_block(range(12, 16))

    nc.compile()
    return nc


def _get_program(loop_n=0):
    key = loop_n
    if key not in _PROG:
        _PROG[key] = _build_program(loop_n)
    return _PROG[key]


def _host_prep(x, Wq, Wk, Wv, Wp, q_gain):
    """Build the 8 per-core input maps: core c -> (kv head c//2, batch c%2)."""
    import ml_dtypes
    bf16 = ml_dtypes.bfloat16

    x = np.ascontiguousarray(x.reshape(B, S, D), dtype=np.float32)
    xTb = [np.ascontiguousarray(x[b].T.astype(bf16)) for b in range(B)]

    inv_freq = 1.0 / (ROPE_BASE ** (np.arange(0, HD, 2, dtype=np.float32) / HD))
    freqs = np.arange(S, dtype=np.float32)[:, None] * inv_freq[None, :]
    cos = np.ascontiguousarray(np.cos(freqs).astype(bf16))   # [S, 64]
    sin = np.ascontiguousarray(np.sin(freqs).astype(bf16))

    r = np.arange(P)[:, None]
    c = np.arange(P)[None, :]
    rmask = np.where(c < r, NEG, 0.0).astype(np.float32)   # [128, 128] tri

    scale = 1.0 / math.sqrt(HD)
    in_maps = []
    for core in range(N_CORES):
        kv = core // 2
        b = core % 2
        h0 = 4 * kv
        WqT = Wq[h0 * HD:(h0 + 4) * HD, :].T             # [D, 512]
        WkT = Wk[kv * HD:(kv + 1) * HD, :].T             # [D, 128]
        WvT = Wv[kv * HD:(kv + 1) * HD, :].T             # [D, 128]
        wcat = np.ascontiguousarray(
            np.concatenate([WqT, WkT, WvT], axis=1).astype(bf16))
        wpT = np.ascontiguousarray(
            Wp[:, h0 * HD:(h0 + 4) * HD].T.astype(bf16))        # [512, D]
        gain = np.tile(np.array(
            [[q_gain[h0] * scale, q_gain[h0 + 1] * scale,
              q_gain[h0 + 2] * scale, q_gain[h0 + 3] * scale, 1.0, EPS]],
            dtype=np.float32), (P, 1))
        in_maps.append({
            "xT": xTb[b],
            "wcat": wcat,
            "wp": wpT,
            "cosd": cos,
            "sind": sin,
            "rmaskd": rmask,
            "gaind": np.ascontiguousarray(gain),
        })
    return in_maps


def kernel(x, Wq, Wk, Wv, Wp, q_gain):
    from concourse.bass_utils import run_bass_kernel_spmd

    nc = _get_program()
    in_maps = _host_prep(x, Wq, Wk, Wv, Wp, q_gain)
    try:
        res = run_bass_kernel_spmd(nc, in_maps, core_ids=list(range(N_CORES)))
    except Exception:
        # one retry: a previous crashed run can leave the exec unit wedged
        res = run_bass_kernel_spmd(nc, in_maps, core_ids=list(range(N_CORES)))
    total = np.zeros((B, S, D), dtype=np.float32)
    for core, r in enumerate(res.results):
        total[core % 2] += r["out"].astype(np.float32)
    return total

